# revision 4
# baseline (speedup 1.0000x reference)
"""Trainium2 Bass kernel for nn_AttentionCell (per-point sparse attention).

Reference computation (per batch b, point n):
  X = input[b, :, n, :]                    # [C=128, K=16]
  qk = W_qk @ X + b_qk                     # [128, 16] -> q = qk[:64], k = qk[64:]
  E  = q^T k                               # [16, 16]
  att = softmax(E / 8, axis=-1)
  w_j = att[j, 0]   (j = 0..15)
  out[b, n, :] = sum_j w_j * (W_v X + b_v)[:, j]
              = W_v (sum_j w_j X[:, j]) + b_v * (sum_j w_j)

Distribution: data-parallel over batch B=16 across 8 cores (2 batches/core).
"""

import sys

sys.path.insert(0, "/opt/trn_rl_repo")

from contextlib import ExitStack

import numpy as np
import ml_dtypes

import concourse.bass as bass
import concourse.tile as tile
from concourse import bacc, mybir
from concourse.bass_utils import run_bass_kernel_spmd

F32 = mybir.dt.float32
F32R = mybir.dt.float32r
BF16 = mybir.dt.bfloat16

B, C, N, K = 16, 128, 1024, 16
QK = 64
VD = 128
N_CORES = 8
B_PER_CORE = B // N_CORES          # 2
PTS = 32                            # points per tile
FREE = PTS * K                      # 512
TILES_PER_BATCH = N // PTS          # 32
SUPER = 4                           # tiles per output super-tile (128 points)

_CACHE = {}


def _build():
    nc = bacc.Bacc("TRN2", target_bir_lowering=False, debug=False,
                   num_devices=N_CORES)

    x_ext = nc.dram_tensor("x", [B_PER_CORE, C, N, K], F32,
                           kind="ExternalInput").ap()
    wqkT_ext = nc.dram_tensor("wqkT", [C, 128], F32, kind="ExternalInput").ap()
    bqk_ext = nc.dram_tensor("bqk", [128, 1], F32, kind="ExternalInput").ap()
    wvT_ext = nc.dram_tensor("wvT", [C, VD], F32, kind="ExternalInput").ap()
    bv_ext = nc.dram_tensor("bv", [1, VD], F32, kind="ExternalInput").ap()
    selz_ext = nc.dram_tensor("selz", [128, 4], BF16, kind="ExternalInput").ap()
    seln_ext = nc.dram_tensor("seln", [128, 4], BF16, kind="ExternalInput").ap()
    ones_ext = nc.dram_tensor("ones", [1, 128], BF16, kind="ExternalInput").ap()
    out_ext = nc.dram_tensor("out", [B_PER_CORE, N, VD], F32,
                             kind="ExternalOutput").ap()

    with tile.TileContext(nc) as tc, ExitStack() as ctx:
        cpool = ctx.enter_context(tc.tile_pool(name="consts", bufs=1))
        xpool = ctx.enter_context(tc.tile_pool(name="x", bufs=3))
        qkpool = ctx.enter_context(tc.tile_pool(name="qk", bufs=2))
        gpool = ctx.enter_context(tc.tile_pool(name="gath", bufs=2))
        zpool = ctx.enter_context(tc.tile_pool(name="z4", bufs=2))
        opool = ctx.enter_context(tc.tile_pool(name="osb", bufs=2))
        ps_qk = ctx.enter_context(tc.tile_pool(name="ps_qk", bufs=2, space="PSUM"))
        ps_g = ctx.enter_context(tc.tile_pool(name="ps_g", bufs=1, space="PSUM"))
        ps_zn = ctx.enter_context(tc.tile_pool(name="ps_zn", bufs=1, space="PSUM"))
        ps_wb = ctx.enter_context(tc.tile_pool(name="ps_wb", bufs=2, space="PSUM"))
        ps_o = ctx.enter_context(tc.tile_pool(name="ps_o", bufs=1, space="PSUM"))

        # ---- constants (loaded once) ----
        wqkT = cpool.tile([C, 128], F32R, tag="wqkT")
        nc.sync.dma_start(wqkT[:], wqkT_ext[:].bitcast(F32R))
        bqk = cpool.tile([128, 1], F32, tag="bqk")
        nc.sync.dma_start(bqk[:], bqk_ext[:])
        wvT = cpool.tile([C, VD], F32, tag="wvT")
        nc.sync.dma_start(wvT[:], wvT_ext[:])
        bv = cpool.tile([1, VD], F32, tag="bv")
        nc.sync.dma_start(bv[:], bv_ext[:])
        selz = cpool.tile([128, 4], BF16, tag="selz")
        nc.sync.dma_start(selz[:], selz_ext[:])
        seln = cpool.tile([128, 4], BF16, tag="seln")
        nc.sync.dma_start(seln[:], seln_ext[:])
        ones = cpool.tile([1, 128], BF16, tag="ones")
        nc.sync.dma_start(ones[:], ones_ext[:])

        # gram psum buffers: zeroed once; gram matmuls only touch rows
        # [32c, 32c+16) so the other rows stay 0 forever (exp(0)=1 times a
        # zero selector row contributes nothing).
        gram_bufs = []
        for i in range(2):
            gb = ps_g.tile([128, PTS * 4], F32, tag=f"g{i}")
            nc.vector.memset(gb[:], 0.0)
            gram_bufs.append(gb)

        for b in range(B_PER_CORE):
            for g in range(TILES_PER_BATCH // SUPER):
                z4 = zpool.tile([C, SUPER * PTS], F32, tag="z4")
                s4 = zpool.tile([1, SUPER * PTS], F32, tag="s4")
                for ti in range(SUPER):
                    t = g * SUPER + ti
                    n0 = t * PTS
                    # ---- load X tile ----
                    xt = xpool.tile([C, FREE], F32R, tag="xt")
                    nc.sync.dma_start(
                        xt[:].rearrange("c (n k) -> c n k", k=K),
                        x_ext[b, :, n0:n0 + PTS, :].bitcast(F32R))

                    # ---- qk projection (f32r, full rate at free=512) ----
                    pqk = ps_qk.tile([128, FREE], F32, tag="pqk")
                    nc.tensor.matmul(pqk[:], wqkT[:], xt[:],
                                     start=True, stop=True)

                    # ---- evacuate with bias, cast bf16; q in parts 0-63 ----
                    qsb = qkpool.tile([128, FREE], BF16, tag="qsb")
                    nc.scalar.activation(qsb[:], pqk[:],
                                         mybir.ActivationFunctionType.Identity,
                                         bias=bqk[:], scale=1.0)
                    # k moved down to partitions 0-63
                    ksb = qkpool.tile([64, FREE], BF16, tag="ksb")
                    nc.vector.tensor_copy(ksb[:], qsb[64:128, :])

                    # ---- per-point gram matmuls E^T = k^T q ----
                    gb = gram_bufs[t % 2]
                    for p in range(PTS):
                        cq = p // 8
                        r = p % 8
                        nc.tensor.matmul(
                            gb[32 * cq:32 * cq + K, r * K:(r + 1) * K],
                            ksb[:, p * K:(p + 1) * K],
                            qsb[0:64, p * K:(p + 1) * K],
                            start=True, stop=True,
                            tile_position=(0, 32 * cq))

                    # ---- exp(E/8) ----
                    ex = gpool.tile([128, PTS * 4], BF16, tag="ex")
                    nc.scalar.activation(ex[:], gb[:],
                                         mybir.ActivationFunctionType.Exp,
                                         bias=0.0, scale=0.125)

                    # ---- Z (denominators) and numerators via selector matmuls
                    zn = ps_zn.tile([4, 2 * PTS * 4], F32, tag="zn")
                    nc.tensor.matmul(zn[:, 0:PTS * 4], selz[:], ex[:],
                                     start=True, stop=True)
                    nc.tensor.matmul(zn[:, PTS * 4:], seln[:], ex[:],
                                     start=True, stop=True)

                    # ---- w = num / Z (thin form), s = sum_j w ----
                    rz = gpool.tile([4, PTS * 4], F32, tag="rz")
                    nc.vector.reciprocal(rz[:], zn[:, 0:PTS * 4])
                    wt = gpool.tile([4, PTS * 4], BF16, tag="wt")
                    nc.vector.tensor_mul(wt[:], zn[:, PTS * 4:], rz[:])
                    st = gpool.tile([4, PTS // 4], F32, tag="st")
                    nc.vector.tensor_reduce(
                        st[:], wt[:].rearrange("c (r j) -> c r j", j=K),
                        axis=mybir.AxisListType.X, op=mybir.AluOpType.add)

                    # ---- shuffle w into a [1, 512] row (point-major) ----
                    # stream c holds points [8c, 8c+8) so the row is a plain
                    # partition-major flatten of wt.
                    wrow = gpool.tile([1, FREE], BF16, tag="wrow")
                    nc.sync.dma_start(wrow[:], wt[:])
                    nc.sync.dma_start(s4[:, ti * PTS:(ti + 1) * PTS], st[:])

                    # ---- broadcast w across partitions via ones matmul ----
                    wb = ps_wb.tile([128, FREE], F32, tag="wb")
                    nc.tensor.matmul(wb[:], ones[:], wrow[:],
                                     start=True, stop=True)

                    # ---- z = sum_j w_j * x_j  (per point) ----
                    xw = qkpool.tile([C, FREE], F32, tag="xw")
                    nc.vector.tensor_mul(xw[:], xt[:].bitcast(F32), wb[:])
                    nc.vector.tensor_reduce(
                        z4[:, ti * PTS:(ti + 1) * PTS],
                        xw[:].rearrange("c (n j) -> c n j", j=K),
                        axis=mybir.AxisListType.X, op=mybir.AluOpType.add)

                # ---- out = z4^T @ W_v^T + s4^T b_v  -> [128 pts, 128 ch] ----
                po = ps_o.tile([SUPER * PTS, VD], F32, tag="po")
                nc.tensor.matmul(po[:], z4[:], wvT[:], start=True, stop=False)
                nc.tensor.matmul(po[:], s4[:], bv[:], start=False, stop=True)
                osb = opool.tile([SUPER * PTS, VD], F32, tag="osb")
                nc.scalar.copy(osb[:], po[:])
                nc.sync.dma_start(
                    out_ext[b, g * SUPER * PTS:(g + 1) * SUPER * PTS, :],
                    osb[:])

    nc.compile()
    return nc


def _host_inputs(input_tensor, W_qk, b_qk, W_v, b_v):
    """Build the per-core input maps (host-side prep is free)."""
    W_qk = np.asarray(W_qk, dtype=np.float32)
    b_qk = np.asarray(b_qk, dtype=np.float32)
    W_v = np.asarray(W_v, dtype=np.float32)
    b_v = np.asarray(b_v, dtype=np.float32)
    x = np.asarray(input_tensor, dtype=np.float32)

    selz = np.zeros((128, 4), dtype=ml_dtypes.bfloat16)
    seln = np.zeros((128, 4), dtype=ml_dtypes.bfloat16)
    for s in range(4):
        selz[32 * s:32 * s + K, s] = 1.0
        seln[32 * s, s] = 1.0
    ones = np.ones((1, 128), dtype=ml_dtypes.bfloat16)

    common = {
        "wqkT": np.ascontiguousarray(W_qk.T),
        "bqk": b_qk.reshape(128, 1).copy(),
        "wvT": np.ascontiguousarray(W_v.T),
        "bv": b_v.reshape(1, VD).copy(),
        "selz": selz,
        "seln": seln,
        "ones": ones,
    }
    in_maps = []
    for ci in range(N_CORES):
        m = dict(common)
        m["x"] = np.ascontiguousarray(x[ci * B_PER_CORE:(ci + 1) * B_PER_CORE])
        in_maps.append(m)
    return in_maps


def _get_graph():
    if "nc" not in _CACHE:
        _CACHE["nc"] = _build()
    return _CACHE["nc"]


def run(input_tensor, W_qk, b_qk, W_v, b_v, **run_kwargs):
    nc = _get_graph()
    in_maps = _host_inputs(input_tensor, W_qk, b_qk, W_v, b_v)
    res = run_bass_kernel_spmd(nc, in_maps, list(range(N_CORES)), **run_kwargs)
    out = np.concatenate([res.results[i]["out"] for i in range(N_CORES)],
                         axis=0)
    return out, res


def kernel(input_tensor, W_qk, b_qk, W_v, b_v, qk_dim=None):
    out, _ = run(input_tensor, W_qk, b_qk, W_v, b_v)
    return out
